# revision 1
# baseline (speedup 1.0000x reference)
"""GCN encoder (3-layer, PyG GCNConv normalize=False + BN eval + ReLU) on 8 trn2 cores.

Strategy (node/dst-sharded, graph-parallel):
  - Nodes are remapped into 8 cores x 49 tiles x 128 slots, balanced by in-degree.
  - Per layer: z = table @ W.T is computed node-sharded on each core, AllGathered
    (bf16) into a full z table in DRAM; each core then gathers z[src] rows for its
    own dst edges with dma_gather and performs the weighted segment-sum as a chain
    of 128-edge one-hot matmuls (scatter matrices precomputed on host, edge
    weights folded in) accumulating into PSUM per 128-node dst tile.
  - Epilogue: per-feature scale/bias (BN folded) + ReLU on DVE, then PE-transpose
    feeds the next layer's z-slice matmul. Layer 3: +b3 only, fp32 output.
"""

import math
from dataclasses import dataclass

import ml_dtypes
import numpy as np

P = 128
HIDDEN = 256


@dataclass
class Cfg:
    n: int = 50000
    e: int = 1600000
    ncores: int = 8
    tiles: int = 49  # dst node tiles of 128 slots per core
    cin: int = 128

    @property
    def slots_per_core(self) -> int:
        return self.tiles * P

    @property
    def total_slots(self) -> int:
        return self.ncores * self.slots_per_core

    @property
    def half(self) -> int:
        # z-table row split for int16 gather indices
        return self.total_slots // 2


CFG = Cfg()


# ---------------------------------------------------------------------------
# Host-side preprocessing
# ---------------------------------------------------------------------------

def _balance_nodes(indeg: np.ndarray, cfg: Cfg) -> np.ndarray:
    """Assign each node a slot in [0, total_slots) so that each 128-slot tile has
    roughly equal total in-degree. Returns slot_of_node [n]."""
    import heapq

    nbins = cfg.ncores * cfg.tiles
    order = np.argsort(-indeg, kind="stable")
    heap = [(0, b) for b in range(nbins)]
    heapq.heapify(heap)
    counts = np.zeros(nbins, dtype=np.int64)
    slot_of = np.empty(cfg.n, dtype=np.int64)
    spill = []
    for v in order:
        load, b = heapq.heappop(heap)
        slot_of[v] = b * P + counts[b]
        counts[b] += 1
        load += int(indeg[v])
        if counts[b] < P:
            heapq.heappush(heap, (load, b))
        else:
            spill.append(b)
    return slot_of


def _prep(cfg: Cfg, x, edge_index, edge_attr, W1, b1, g1, beta1, m1, v1,
          W2, b2, g2, beta2, m2, v2, W3, b3):
    bf16 = ml_dtypes.bfloat16
    n, e = cfg.n, cfg.e
    src = np.asarray(edge_index[0], dtype=np.int64)
    dst = np.asarray(edge_index[1], dtype=np.int64)
    ew = np.asarray(edge_attr, dtype=np.float32).mean(axis=1)

    indeg = np.bincount(dst, minlength=n)
    slot_of = _balance_nodes(indeg, cfg)

    sslot = slot_of[src]
    dslot = slot_of[dst]
    ebin = dslot // P  # 0 .. ncores*tiles-1
    is_hi = sslot >= cfg.half

    # group key: (bin, lo/hi); lo first
    nbins = cfg.ncores * cfg.tiles
    key = ebin * 2 + is_hi.astype(np.int64)
    order = np.argsort(key, kind="stable")
    key_s = key[order]
    counts_g = np.bincount(key_s, minlength=nbins * 2)
    gstart = np.zeros(nbins * 2, dtype=np.int64)
    gstart[1:] = np.cumsum(counts_g)[:-1]
    rank = np.arange(e, dtype=np.int64) - gstart[key_s]  # rank within group

    lo_counts = counts_g[0::2].reshape(cfg.ncores, cfg.tiles)
    hi_counts = counts_g[1::2].reshape(cfg.ncores, cfg.tiles)
    ct_lo = int(math.ceil(lo_counts.max() / P))
    ct_hi = int(math.ceil(hi_counts.max() / P))
    ct = ct_lo + ct_hi

    # per-edge destination in padded chunk arrays
    e_bin = key_s // 2
    e_hi = key_s % 2
    e_core = e_bin // cfg.tiles
    e_tile = e_bin % cfg.tiles
    e_chunk = rank // P + e_hi * ct_lo
    e_lane = rank % P
    e_sslot = slot_of[src][order] - e_hi * cfg.half  # index into z half-table
    e_dlocal = (dslot[order]) % P  # dst slot within tile
    e_w12 = ew[order].astype(np.float32)

    # IDX[core, tile, chunk, lane] int16, pad = 0 (valid row, weight 0)
    idx = np.zeros((cfg.ncores, cfg.tiles, ct, P), dtype=np.int16)
    idx[e_core, e_tile, e_chunk, e_lane] = e_sslot.astype(np.int16)

    # Scatter matrices, stored transposed for matmul lhsT:
    # MT[core, tile, lane(=edge, partition dim), chunk, dstslot] (lane-major for
    # contiguous per-partition DMA)
    mt12f = np.zeros((cfg.ncores, cfg.tiles, P, ct, P), dtype=np.float32)
    mt3f = np.zeros((cfg.ncores, cfg.tiles, P, ct, P), dtype=np.float32)
    np.add.at(mt12f, (e_core, e_tile, e_lane, e_chunk, e_dlocal), e_w12)
    np.add.at(mt3f, (e_core, e_tile, e_lane, e_chunk, e_dlocal), 1.0)
    mt12 = mt12f.astype(bf16)
    mt3 = mt3f.astype(bf16)
    del mt12f, mt3f

    # gather-call index layout: per (tile, group) block of ctg*8 columns;
    # value at (partition p, col s) = idx_linear[s*16 + p%16], replicated x8.
    idx_sb = np.zeros((cfg.ncores, P, cfg.tiles * ct * 8), dtype=np.int16)
    for g, ctg, off in ((0, ct_lo, 0), (1, ct_hi, ct_lo)):
        if ctg == 0:
            continue
        blk = idx[:, :, off:off + ctg, :].reshape(cfg.ncores, cfg.tiles, ctg * P)
        # columns for this group within each tile block
        cols = blk.reshape(cfg.ncores, cfg.tiles, ctg * 8, 16)
        colbase = off * 8
        for tcol in range(ctg * 8):
            dst_col = np.arange(cfg.tiles) * (ct * 8) + colbase + tcol
            idx_sb[:, :16, dst_col] = cols[:, :, tcol, :].transpose(0, 2, 1)
    idx_sb[:, 16:, :] = np.tile(idx_sb[:, :16, :], (1, 7, 1))

    # x^T per core [cin, slots_per_core], bf16; pad slots -> 0
    node_of_slot = np.full(cfg.total_slots, -1, dtype=np.int64)
    node_of_slot[slot_of] = np.arange(n)
    xt = np.zeros((cfg.ncores, cfg.cin, cfg.slots_per_core), dtype=bf16)
    xf = np.asarray(x, dtype=np.float32)
    for c in range(cfg.ncores):
        sl = node_of_slot[c * cfg.slots_per_core:(c + 1) * cfg.slots_per_core]
        valid = sl >= 0
        buf = np.zeros((cfg.slots_per_core, cfg.cin), dtype=np.float32)
        buf[valid] = xf[sl[valid]]
        xt[c] = buf.T.astype(bf16)

    # weights / epilogue params
    eps = 1e-5
    s1 = (np.asarray(g1) / np.sqrt(np.asarray(v1) + eps)).astype(np.float32)
    t1 = (np.asarray(beta1) + (np.asarray(b1) - np.asarray(m1)) * s1).astype(np.float32)
    s2 = (np.asarray(g2) / np.sqrt(np.asarray(v2) + eps)).astype(np.float32)
    t2 = (np.asarray(beta2) + (np.asarray(b2) - np.asarray(m2)) * s2).astype(np.float32)

    def rep(v):
        return np.broadcast_to(np.asarray(v, np.float32), (P, HIDDEN)).copy()

    w1t = np.asarray(W1, np.float32).T.astype(bf16)  # [cin, 256]
    w2t = np.asarray(W2, np.float32).T.reshape(2, P, HIDDEN).astype(bf16)
    w3t = np.asarray(W3, np.float32).T.reshape(2, P, HIDDEN).astype(bf16)

    in_maps = []
    for c in range(cfg.ncores):
        in_maps.append({
            "xt": np.ascontiguousarray(xt[c]),
            "idx": np.ascontiguousarray(idx_sb[c]),
            "mt12": np.ascontiguousarray(
                mt12[c].reshape(cfg.tiles, P, ct * P)),
            "mt3": np.ascontiguousarray(mt3[c].reshape(cfg.tiles, P, ct * P)),
            "w1t": w1t,
            "w2t": w2t,
            "w3t": w3t,
            "sc1": rep(s1), "bi1": rep(t1),
            "sc2": rep(s2), "bi2": rep(t2),
            "bi3": rep(np.asarray(b3, np.float32)),
        })
    return in_maps, node_of_slot, ct_lo, ct_hi


# ---------------------------------------------------------------------------
# Bass program
# ---------------------------------------------------------------------------

def _build(cfg: Cfg, ct_lo: int, ct_hi: int):
    import concourse.mybir as mybir
    import concourse.tile as tile
    from concourse import bacc
    from concourse.masks import make_identity

    ct = ct_lo + ct_hi
    T = cfg.tiles
    SPC = cfg.slots_per_core
    DT = mybir.dt
    nc = bacc.Bacc("TRN2", target_bir_lowering=False, debug=False,
                   num_devices=cfg.ncores, num_swdge_queues=4)

    xt_d = nc.declare_dram_parameter("xt", [cfg.cin, SPC], DT.bfloat16, isOutput=False)
    idx_d = nc.declare_dram_parameter("idx", [P, T * ct * 8], DT.int16, isOutput=False)
    mt12_d = nc.declare_dram_parameter("mt12", [T, P, ct * P], DT.bfloat16, isOutput=False)
    mt3_d = nc.declare_dram_parameter("mt3", [T, P, ct * P], DT.bfloat16, isOutput=False)
    w1t_d = nc.declare_dram_parameter("w1t", [cfg.cin, HIDDEN], DT.bfloat16, isOutput=False)
    w2t_d = nc.declare_dram_parameter("w2t", [2, P, HIDDEN], DT.bfloat16, isOutput=False)
    w3t_d = nc.declare_dram_parameter("w3t", [2, P, HIDDEN], DT.bfloat16, isOutput=False)
    sc1_d = nc.declare_dram_parameter("sc1", [P, HIDDEN], DT.float32, isOutput=False)
    bi1_d = nc.declare_dram_parameter("bi1", [P, HIDDEN], DT.float32, isOutput=False)
    sc2_d = nc.declare_dram_parameter("sc2", [P, HIDDEN], DT.float32, isOutput=False)
    bi2_d = nc.declare_dram_parameter("bi2", [P, HIDDEN], DT.float32, isOutput=False)
    bi3_d = nc.declare_dram_parameter("bi3", [P, HIDDEN], DT.float32, isOutput=False)
    out_d = nc.declare_dram_parameter("out", [SPC, HIDDEN], DT.float32, isOutput=True)

    zslice = [nc.dram_tensor(f"zslice{l}", [SPC, HIDDEN], DT.bfloat16)
              for l in range(3)]
    zfull = [
        nc.dram_tensor(f"zfull{l}", [cfg.total_slots, HIDDEN], DT.bfloat16,
                       addr_space="Shared")
        for l in range(3)
    ]
    groups = [list(range(cfg.ncores))]

    with tile.TileContext(nc) as tc:
        with (
            tc.tile_pool(name="const", bufs=1) as const_pool,
            tc.tile_pool(name="mpool", bufs=4) as m_pool,
            tc.tile_pool(name="gpool", bufs=6) as g_pool,
            tc.tile_pool(name="zpool", bufs=3) as z_pool,
            tc.tile_pool(name="epool", bufs=2) as e_pool,
            tc.tile_pool(name="tpool", bufs=2) as t_pool,
            tc.tile_pool(name="agg_ps", bufs=4, space="PSUM") as agg_psum,
            tc.tile_pool(name="tr_ps", bufs=2, space="PSUM") as tr_psum,
            tc.tile_pool(name="z_ps", bufs=2, space="PSUM") as z_psum,
        ):
            # persistent tiles
            idx_sb = const_pool.tile([P, T * ct * 8], DT.int16)
            nc.sync.dma_start(idx_sb[:], idx_d[:])
            xt_sb = const_pool.tile([cfg.cin, SPC], DT.bfloat16)
            nc.sync.dma_start(xt_sb[:], xt_d[:])
            w1t_sb = const_pool.tile([cfg.cin, HIDDEN], DT.bfloat16)
            nc.sync.dma_start(w1t_sb[:], w1t_d[:])
            w2t_sb = const_pool.tile([P, 2, HIDDEN], DT.bfloat16)
            nc.sync.dma_start(w2t_sb[:], w2t_d[:].rearrange("h p n -> p h n"))
            w3t_sb = const_pool.tile([P, 2, HIDDEN], DT.bfloat16)
            nc.sync.dma_start(w3t_sb[:], w3t_d[:].rearrange("h p n -> p h n"))
            sc_sb, bi_sb = [], []
            for i, d in enumerate((sc1_d, sc2_d)):
                t_ = const_pool.tile([P, HIDDEN], DT.float32, tag=f"sc{i}")
                nc.sync.dma_start(t_[:], d[:])
                sc_sb.append(t_)
            for i, d in enumerate((bi1_d, bi2_d, bi3_d)):
                t_ = const_pool.tile([P, HIDDEN], DT.float32, tag=f"bi{i}")
                nc.sync.dma_start(t_[:], d[:])
                bi_sb.append(t_)
            ident = const_pool.tile([P, P], DT.bfloat16)
            make_identity(nc, ident[:])

            # ---------------- layer-1 z slices ----------------
            for t in range(T):
                ps = z_psum.tile([P, HIDDEN], DT.float32, tag="zps")
                nc.tensor.matmul(ps[:], xt_sb[:, t * P:(t + 1) * P], w1t_sb[:],
                                 start=True, stop=True)
                zt = z_pool.tile([P, HIDDEN], DT.bfloat16, tag="z1")
                nc.vector.tensor_copy(zt[:], ps[:])
                nc.sync.dma_start(zslice[0][t * P:(t + 1) * P, :], zt[:])
            nc.gpsimd.collective_compute(
                "AllGather", mybir.AluOpType.bypass, replica_groups=groups,
                ins=[zslice[0][:]], outs=[zfull[0][:]])

            # ---------------- 3 conv layers ----------------
            for l in range(3):
                mt_d = mt12_d if l < 2 else mt3_d
                zf = zfull[l]
                for t in range(T):
                    glo = g_pool.tile([P, ct_lo, HIDDEN], DT.bfloat16, tag="glo")
                    nc.gpsimd.dma_gather(
                        glo[:], zf[0:cfg.half, :],
                        idx_sb[:, t * ct * 8: t * ct * 8 + ct_lo * 8],
                        ct_lo * P, ct_lo * P, HIDDEN, single_packet=False,
                        queue_num=(2 * t) % 4)
                    ghi = g_pool.tile([P, ct_hi, HIDDEN], DT.bfloat16, tag="ghi")
                    nc.gpsimd.dma_gather(
                        ghi[:], zf[cfg.half:, :],
                        idx_sb[:, t * ct * 8 + ct_lo * 8: (t + 1) * ct * 8],
                        ct_hi * P, ct_hi * P, HIDDEN, single_packet=False,
                        queue_num=(2 * t + 1) % 4)
                    mt_sb = m_pool.tile([P, ct * P], DT.bfloat16)
                    nc.sync.dma_start(mt_sb[:], mt_d[t])

                    ps = agg_psum.tile([P, HIDDEN], DT.float32)
                    for k in range(ct):
                        g = glo[:, k, :] if k < ct_lo else ghi[:, k - ct_lo, :]
                        nc.tensor.matmul(ps[:], mt_sb[:, k * P:(k + 1) * P], g,
                                         start=(k == 0), stop=(k == ct - 1))

                    if l < 2:
                        tmp = e_pool.tile([P, HIDDEN], DT.float32)
                        nc.vector.tensor_tensor(
                            out=tmp[:], in0=ps[:], in1=sc_sb[l][:],
                            op=mybir.AluOpType.mult)
                        nc.vector.tensor_tensor(
                            out=tmp[:], in0=tmp[:], in1=bi_sb[l][:],
                            op=mybir.AluOpType.add)
                        relu = z_pool.tile([P, HIDDEN], DT.bfloat16, tag="relu")
                        nc.vector.tensor_scalar_max(relu[:], tmp[:], 0.0)
                        # z_{l+1} slice = relu @ W_{l+1}.T (needs relu^T tiles)
                        wnext = w2t_sb if l == 0 else w3t_sb
                        zps = z_psum.tile([P, HIDDEN], DT.float32, tag="zps")
                        for h in range(2):
                            tp = tr_psum.tile([P, P], DT.bfloat16)
                            nc.tensor.transpose(
                                tp[:], relu[:, h * P:(h + 1) * P], ident[:])
                            tt = t_pool.tile([P, P], DT.bfloat16)
                            nc.vector.tensor_copy(tt[:], tp[:])
                            nc.tensor.matmul(zps[:], tt[:], wnext[:, h, :],
                                             start=(h == 0), stop=(h == 1))
                        zn = z_pool.tile([P, HIDDEN], DT.bfloat16, tag="zn")
                        nc.vector.tensor_copy(zn[:], zps[:])
                        nc.sync.dma_start(
                            zslice[l + 1][t * P:(t + 1) * P, :], zn[:])
                    else:
                        ot = e_pool.tile([P, HIDDEN], DT.float32, tag="out")
                        nc.vector.tensor_tensor(
                            out=ot[:], in0=ps[:], in1=bi_sb[2][:],
                            op=mybir.AluOpType.add)
                        nc.sync.dma_start(out_d[t * P:(t + 1) * P, :], ot[:])
                if l < 2:
                    nc.gpsimd.collective_compute(
                        "AllGather", mybir.AluOpType.bypass,
                        replica_groups=groups,
                        ins=[zslice[l + 1][:]], outs=[zfull[l + 1][:]])
    nc.compile()
    return nc


# ---------------------------------------------------------------------------
# Entry point
# ---------------------------------------------------------------------------

LAST_RESULTS = None  # BassKernelResults of the most recent _run (for profiling)


def _run(cfg: Cfg, inputs: dict, trace: bool = False,
         trace_cores=None) -> np.ndarray:
    global LAST_RESULTS
    from concourse.bass_utils import run_bass_kernel_spmd

    in_maps, node_of_slot, ct_lo, ct_hi = _prep(cfg, **inputs)
    nc = _build(cfg, ct_lo, ct_hi)
    kr = run_bass_kernel_spmd(nc, in_maps, list(range(cfg.ncores)), trace=trace,
                              trace_cores=trace_cores)
    LAST_RESULTS = kr
    res = kr.results
    out = np.empty((cfg.n, HIDDEN), dtype=np.float32)
    full = np.concatenate([res[c]["out"] for c in range(cfg.ncores)], axis=0)
    valid = node_of_slot >= 0
    out[node_of_slot[valid]] = full[valid]
    return out


def kernel(**inputs) -> np.ndarray:
    return _run(CFG, inputs)

